# revision 27
# baseline (speedup 1.0000x reference)
"""GQA attention (SEQ=2048, DIM=4096, 32 Q heads / 8 KV heads, head_dim=128),
tensor-parallel over heads across 8 NeuronCores.

Each core owns 4 Q heads + 1 KV head: wq/wk/wv split column-wise, wo split
row-wise; each core produces a partial (2048, 4096) output that the host sums
(the all-reduce of row-parallel wo).

Per-core kernel (all matmul operands bf16, fp32 PSUM accumulate):
  A) QKV projections per 512-seq block: x slab resident in SBUF (chunked
     9-deep rotation), two passes over the slab (q0,q1,K then q2,q3,V) so only
     3 PSUM accumulators are live per pass and the 2-buf rotation never stalls
     on eviction; RoPE on PSUM eviction (ACT half-swap + DVE mul/add). V^T is
     evicted to SBUF (DVE); its PE transposes to natural layout are deferred
     to the matching phase-B header.
  B) Flash attention, flattened over (head, key-block) per 512-query block:
     for qb>=2, score pairs go into wide [128,1024] PSUM tiles (one exp covers
     2 blocks, halving ACT instruction overhead); qb<=1 use narrow [128,512]
     tiles (lower exp latency for the short streams). Diagonal blocks are
     causally trimmed for qb>=1 (S/exp/D/AV touch only queries >= the block
     diagonal), with the j=0 diagonal drained last so the PSUM accumulation
     stop lands on a full-range matmul; qb=0 uses the full staircase mask so
     masked es is exactly 0 and full-range drains are valid. D is broadcast
     across partitions via an all-ones stationary matmul; D/AV drains trail
     the score stream by LAG=6 blocks across head boundaries; normalize =
     fast-approx reciprocal + mul on DVE, off the PE critical path.
  C) out = O^T.T @ wo accumulated over the 4 heads, bf16 partials to DRAM;
     issued interleaved with phase B (B0,B1,C0,B2,C1,B3,C2,C3) so normalize
     tails hide under the next query block's score stream.

One unified PSUM pool: s2 [128,1024]x2 + b,c [128,512]x2 = exactly 8 banks,
spanning all phases (no pool-boundary drain stalls). Phase A's K/V
accumulators share the s2 tag with phase B's score tiles (both evict early),
q0-q3 share b/c with the header transposes, D and phase C's o_ps share b.
"""

import numpy as np
import ml_dtypes

import concourse.bacc as bacc
import concourse.tile as tile
from concourse import mybir
from concourse.bass_utils import run_bass_kernel_spmd

F32 = mybir.dt.float32
F32R = mybir.dt.float32r
BF16 = mybir.dt.bfloat16
NPBF = ml_dtypes.bfloat16

DIM = 4096
SEQ = 2048
HEAD_DIM = 128
N_CORES = 8
QH = 4              # q heads per core
QS = QH * HEAD_DIM  # 512: wq column slice per core
NKT = DIM // 128    # 32 contraction tiles
NSB = SEQ // 512    # 4 sequence blocks
SCALE = 1.0 / float(np.sqrt(HEAD_DIM))
NEG = -1e9
LAG = 6             # D/AV drains trail the score stream by LAG blocks


def build_nc():
    nc = bacc.Bacc(trn_type="TRN2")

    xT = nc.declare_dram_parameter("xT", [DIM, SEQ], BF16, isOutput=False)
    wq = nc.declare_dram_parameter("wq", [DIM, QS], BF16, isOutput=False)
    wk = nc.declare_dram_parameter("wk", [DIM, HEAD_DIM], BF16, isOutput=False)
    wv = nc.declare_dram_parameter("wv", [DIM, HEAD_DIM], BF16, isOutput=False)
    wo = nc.declare_dram_parameter("wo", [QS, DIM], BF16, isOutput=False)
    cosT = nc.declare_dram_parameter("cosT", [HEAD_DIM, SEQ], F32, isOutput=False)
    sinTs = nc.declare_dram_parameter("sinTs", [HEAD_DIM, SEQ], F32, isOutput=False)
    stair = nc.declare_dram_parameter("stair", [128, 128], F32, isOutput=False)
    stair9 = nc.declare_dram_parameter("stair9", [128, 896], F32, isOutput=False)
    ident = nc.declare_dram_parameter("ident", [128, 128], F32R, isOutput=False)
    ones128 = nc.declare_dram_parameter("ones128", [128, 128], BF16, isOutput=False)
    out = nc.declare_dram_parameter("out", [SEQ, DIM], BF16, isOutput=True)

    xT_r = xT.rearrange("(t p) s -> p t s", p=128)
    wq_r = wq.rearrange("(t p) m -> p t m", p=128)
    wk_r = wk.rearrange("(t p) m -> p t m", p=128)
    wv_r = wv.rearrange("(t p) m -> p t m", p=128)
    wo_r = wo.rearrange("(h p) n -> p h n", p=128)

    with tile.TileContext(nc) as tc:
        with (
            tc.tile_pool(name="persist", bufs=1) as persist,
            tc.tile_pool(name="xc", bufs=9) as xcp,
            tc.tile_pool(name="cs", bufs=2) as csp,
            tc.tile_pool(name="rtmp", bufs=2) as rtp,
            tc.tile_pool(name="vtsb", bufs=4) as vtp,
            tc.tile_pool(name="esp", bufs=6) as esp,
            tc.tile_pool(name="otp", bufs=2) as otp,
            tc.tile_pool(name="rdp", bufs=2) as rdp,
            tc.tile_pool(name="esu", bufs=6) as esup,
            tc.tile_pool(name="obp", bufs=4) as obp,
            tc.tile_pool(name="ps", bufs=2, space="PSUM") as ps,
        ):
            # small constants: tiles declared here, DMAs deferred into the
            # first seq block's chunk stream (they are phase-B-only inputs
            # and must not delay the first projection matmuls)
            stair_sb = persist.tile([128, 128], F32)
            stair9_sb = persist.tile([128, 896], F32)
            ident_f32r = persist.tile([128, 128], F32R)
            ones_sb = persist.tile([128, 128], BF16)

            # resident weights
            wq_sb = persist.tile([128, NKT, QS], BF16)
            wk_sb = persist.tile([128, NKT, HEAD_DIM], BF16)
            wv_sb = persist.tile([128, NKT, HEAD_DIM], BF16)
            wo_sb = persist.tile([128, QH, DIM], BF16)

            # per-seq-block activation outputs
            qTb = [persist.tile([128, QH, 512], BF16, name=f"qTb{sb}")
                   for sb in range(NSB)]
            kTb = [persist.tile([128, 512], BF16, name=f"kTb{sb}")
                   for sb in range(NSB)]
            vNb = [persist.tile([128, 4, 128], BF16, name=f"vNb{sb}")
                   for sb in range(NSB)]
            vt_sbs = [None] * NSB  # V^T staged in SBUF, transposed in B headers

            def rope(dst, src_ps, cos_t, sin_t):
                # half-swap via ACT (PSUM->SBUF cross-partition copies are
                # allowed); muls/add on DVE. sin_t has the rotate_half sign
                # folded in (rows 0:64 negated on host).
                vr = rtp.tile([128, 512], F32, tag="vr", name="vr")
                nc.scalar.copy(vr[0:64, :], src_ps[64:128, :])
                nc.vector.tensor_copy(vr[64:128, :], src_ps[0:64, :])
                u = rtp.tile([128, 512], F32, tag="u", name="u")
                nc.vector.tensor_mul(u, vr, sin_t)
                t2 = rtp.tile([128, 512], F32, tag="t2", name="t2")
                nc.vector.tensor_mul(t2, src_ps, cos_t)
                nc.vector.tensor_add(dst, t2, u)

            # ---------------- Phase A: projections + RoPE ----------------
            for sb in range(NSB):
                ss = slice(sb * 512, (sb + 1) * 512)
                cos_t = csp.tile([128, 512], F32, tag="cos", name="cos")
                sin_t = csp.tile([128, 512], F32, tag="sin", name="sin")
                if sb > 0:
                    nc.sync.dma_start(out=cos_t, in_=cosT[:, ss])
                    nc.sync.dma_start(out=sin_t, in_=sinTs[:, ss])

                xcs = []
                for c in range(8):
                    xc = xcp.tile([128, 4, 512], BF16, tag="xc",
                                  name=f"xc{sb}_{c}")
                    if sb == 0 and c <= 1:
                        # kt-granular first chunks: the projection stream
                        # ramps with the DMA engines instead of waiting for
                        # whole 1.25MB chunk groups
                        for i in range(4):
                            kt = c * 4 + i
                            nc.sync.dma_start(out=wq_sb[:, kt:kt + 1, :],
                                              in_=wq_r[:, kt:kt + 1, :])
                            nc.sync.dma_start(out=wk_sb[:, kt:kt + 1, :],
                                              in_=wk_r[:, kt:kt + 1, :])
                            nc.sync.dma_start(out=wv_sb[:, kt:kt + 1, :],
                                              in_=wv_r[:, kt:kt + 1, :])
                            nc.sync.dma_start(out=xc[:, i:i + 1, :],
                                              in_=xT_r[:, kt:kt + 1, ss])
                        xcs.append(xc)
                        continue
                    if sb == 0:
                        nc.sync.dma_start(
                            out=wq_sb[:, c * 4:(c + 1) * 4, :],
                            in_=wq_r[:, c * 4:(c + 1) * 4, :],
                        )
                        nc.sync.dma_start(
                            out=wk_sb[:, c * 4:(c + 1) * 4, :],
                            in_=wk_r[:, c * 4:(c + 1) * 4, :],
                        )
                        nc.sync.dma_start(
                            out=wv_sb[:, c * 4:(c + 1) * 4, :],
                            in_=wv_r[:, c * 4:(c + 1) * 4, :],
                        )
                    nc.sync.dma_start(out=xc, in_=xT_r[:, c * 4:(c + 1) * 4, ss])
                    xcs.append(xc)
                    if sb == 0 and c == 5:
                        # rope tables needed at end of pass 1 (~8us away)
                        nc.sync.dma_start(out=cos_t, in_=cosT[:, ss])
                        nc.sync.dma_start(out=sin_t, in_=sinTs[:, ss])
                    if sb == 0 and c == 6:
                        # phase-B constants, needed only after phase A
                        nc.sync.dma_start(out=stair_sb, in_=stair[:, :])
                        nc.sync.dma_start(out=stair9_sb, in_=stair9[:, :])
                        nc.sync.dma_start(out=ident_f32r, in_=ident[:, :])
                        nc.sync.dma_start(out=ones_sb, in_=ones128[:, :])
                nc.sync.dma_start(out=wo_sb[:, sb, :], in_=wo_r[:, sb, :])

                # pass 1: q heads 0,1 + K (K on tag s2 so phase B's score
                # tiles conflict only with early-evicted projections)
                a_ps = ps.tile([128, 512], F32, tag="b", name="aps")
                b_ps = ps.tile([128, 512], F32, tag="c", name="bps")
                c_ps = ps.tile([128, 512], F32, tag="s2", name="cps")
                for c in range(8):
                    for i in range(4):
                        kt = c * 4 + i
                        st, sp = (kt == 0), (kt == NKT - 1)
                        xt = xcs[c][:, i, :]
                        nc.tensor.matmul(a_ps, wq_sb[:, kt, 0:128], xt,
                                         start=st, stop=sp)
                        nc.tensor.matmul(b_ps, wq_sb[:, kt, 128:256], xt,
                                         start=st, stop=sp)
                        nc.tensor.matmul(c_ps, wk_sb[:, kt, :], xt,
                                         start=st, stop=sp)
                rope(qTb[sb][:, 0, :], a_ps, cos_t, sin_t)
                rope(qTb[sb][:, 1, :], b_ps, cos_t, sin_t)
                rope(kTb[sb], c_ps, cos_t, sin_t)

                # pass 2: q heads 2,3 + V
                d_ps = ps.tile([128, 512], F32, tag="b", name="dps")
                e_ps = ps.tile([128, 512], F32, tag="c", name="eps")
                f_ps = ps.tile([128, 512], F32, tag="s2", name="fps")
                for c in range(8):
                    for i in range(4):
                        kt = c * 4 + i
                        st, sp = (kt == 0), (kt == NKT - 1)
                        xt = xcs[c][:, i, :]
                        nc.tensor.matmul(d_ps, wq_sb[:, kt, 256:384], xt,
                                         start=st, stop=sp)
                        nc.tensor.matmul(e_ps, wq_sb[:, kt, 384:512], xt,
                                         start=st, stop=sp)
                        nc.tensor.matmul(f_ps, wv_sb[:, kt, :], xt,
                                         start=st, stop=sp)
                vt_sb = vtp.tile([128, 512], F32R, tag="vt", name=f"vt{sb}")
                nc.vector.tensor_copy(vt_sb, f_ps)
                vt_sbs[sb] = vt_sb
                rope(qTb[sb][:, 2, :], d_ps, cos_t, sin_t)
                rope(qTb[sb][:, 3, :], e_ps, cos_t, sin_t)

            # ---------------- Phase B: attention per query block ----------------
            ots = [[None] * QH, [None] * QH]  # double-buffered across qb

            def attention(qb):
                # header: V transposes for this qb's diagonal KV tile (their
                # inputs have been ready since phase A)
                for j in range(4):
                    vt_ps = ps.tile([128, 128], F32R, tag="b", name="vtp")
                    nc.tensor.transpose(
                        vt_ps, vt_sbs[qb][:, j * 128:(j + 1) * 128], ident_f32r
                    )
                    nc.scalar.copy(vNb[qb][:, j, :], vt_ps)

                n_kb = 4 * qb + 4
                # drain schedule per head: full blocks in order, then diagonal
                # j=1..3 (trimmed), then j=0 last (full range, carries stop).
                # For qb==0 there is no leading full block, so j=0 is split
                # into [0,128) start+stop and [128,512) stop.
                dq = []  # (h, kb, lo, start, stop, head_last)
                for h in range(QH):
                    items = []
                    if qb == 0:
                        # untrimmed, in order: es is exactly 0 in the masked
                        # region (staircase + exp underflow), so full-range
                        # drains with a single leading start are correct
                        for kb in range(4):
                            items.append((kb, 0, kb == 0, kb == 3))
                    else:
                        for kb in range(4 * qb):
                            items.append((kb, 0, kb == 0, False))
                        items.append((4 * qb + 1, 128, False, False))
                        items.append((4 * qb + 2, 256, False, False))
                        items.append((4 * qb + 3, 384, False, False))
                        items.append((4 * qb, 0, False, True))
                    for idx, it in enumerate(items):
                        dq.append((h, it, idx == len(items) - 1))

                d_ps_h = [None] * QH
                ot_ps_h = [None] * QH
                esw = {}
                state = {"dqi": 0, "issued": 0}

                def drain_ready():
                    h, (kb, rng, st, sp), head_last = dq[state["dqi"]]
                    return (h, kb) in esw

                def do_drain():
                    h, (kb, rng, st, sp), head_last = dq[state["dqi"]]
                    state["dqi"] += 1
                    if d_ps_h[h] is None:
                        d_ps_h[h] = ps.tile([128, 512], F32, tag="b",
                                            name=f"dq{qb}_{h}")
                        ot_ps_h[h] = ps.tile([128, 512], F32, tag="c",
                                             name=f"oq{qb}_{h}")
                    lo, hi = rng if isinstance(rng, tuple) else (rng, 512)
                    es, off = esw[(h, kb)]
                    mv = es[:, off + lo: off + hi]
                    full = kb < 4 * qb
                    if full and kb % 2 == 1:
                        # pair-summed D: one matmul covers blocks kb-1, kb
                        # (the DVE add was issued with the pair's exp);
                        # start flag carried by the first pair
                        nc.tensor.matmul(d_ps_h[h], ones_sb, esw[(h, kb,
                                                                  "sum")],
                                         start=(kb == 1), stop=False)
                    elif not full:
                        # diagonal blocks: per-block D (trimmed, stop on j0)
                        nc.tensor.matmul(d_ps_h[h][:, lo:hi], ones_sb, mv,
                                         start=st, stop=sp)
                    nc.tensor.matmul(ot_ps_h[h][:, lo:hi],
                                     vNb[kb // 4][:, kb % 4, :], mv,
                                     start=st, stop=sp)
                    if head_last:
                        rd = rdp.tile([128, 512], F32, tag="rd", name="rd")
                        nc.vector.reciprocal_approx_fast(rd, d_ps_h[h])
                        ot = otp.tile([128, 512], BF16, tag=f"ot{h}",
                                      name=f"ot{h}")
                        nc.vector.tensor_mul(ot, ot_ps_h[h], rd)
                        ots[qb % 2][h] = ot

                wide = qb >= 2
                for h in range(QH):
                    if wide:
                        for p in range(n_kb // 2):
                            sw = ps.tile([128, 1024], F32, tag="s2", name="sw")
                            for half, kb in enumerate((2 * p, 2 * p + 1)):
                                j = kb - 4 * qb
                                lo = j * 128 if j > 0 else 0
                                nc.tensor.matmul(
                                    sw[:, half * 512 + lo:(half + 1) * 512],
                                    kTb[kb // 4][:, (kb % 4) * 128:
                                                 (kb % 4 + 1) * 128],
                                    qTb[qb][:, h, lo:512],
                                    start=True, stop=True,
                                )
                                if j >= 0:
                                    dst = sw[:, half * 512 + j * 128:
                                             half * 512 + (j + 1) * 128]
                                    nc.vector.tensor_add(dst, dst,
                                                         stair_sb[:, :])
                            jA = max(0, 2 * p - 4 * qb) * 128
                            es = esp.tile([128, 1024], BF16, tag="es",
                                          name="es")
                            nc.scalar.activation(
                                es[:, jA:], sw[:, jA:],
                                mybir.ActivationFunctionType.Exp, scale=SCALE,
                            )
                            esw[(h, 2 * p)] = (es, 0)
                            esw[(h, 2 * p + 1)] = (es, 512)
                            if 2 * p + 1 < 4 * qb:
                                esum = esup.tile([128, 512], BF16, tag="eu",
                                                 name="esum")
                                nc.vector.tensor_add(esum, es[:, 0:512],
                                                     es[:, 512:1024])
                                esw[(h, 2 * p + 1, "sum")] = esum
                            state["issued"] += 2
                            while (state["dqi"] < len(dq)
                                   and state["dqi"] < state["issued"] - LAG
                                   and drain_ready()):
                                do_drain()
                    else:
                        # narrow tiles: shorter exp latency for the short
                        # qb=0/1 streams
                        for kb in range(n_kb):
                            j = kb - 4 * qb
                            lo = j * 128 if (j > 0 and qb > 0) else 0
                            s_ps = ps.tile([128, 512], F32, tag="s2",
                                           name="snp")
                            nc.tensor.matmul(
                                s_ps[:, lo:512],
                                kTb[kb // 4][:, (kb % 4) * 128:
                                             (kb % 4 + 1) * 128],
                                qTb[qb][:, h, lo:512],
                                start=True, stop=True,
                            )
                            if j >= 0:
                                if qb == 0:
                                    # full staircase: masked es exactly 0
                                    nc.vector.tensor_add(
                                        s_ps, s_ps,
                                        stair9_sb[:, 384 - 128 * j:
                                                  896 - 128 * j],
                                    )
                                else:
                                    dst = s_ps[:, j * 128:(j + 1) * 128]
                                    nc.vector.tensor_add(dst, dst,
                                                         stair_sb[:, :])
                            es = esp.tile([128, 512], BF16, tag="es",
                                          name="es")
                            nc.scalar.activation(
                                es[:, lo:], s_ps[:, lo:],
                                mybir.ActivationFunctionType.Exp,
                                scale=SCALE,
                            )
                            esw[(h, kb)] = (es, 0)
                            if kb < 4 * qb and kb % 2 == 1:
                                esum = esup.tile([128, 512], BF16, tag="eu",
                                                 name="esum")
                                nc.vector.tensor_add(
                                    esum, esw[(h, kb - 1)][0], es)
                                esw[(h, kb, "sum")] = esum
                            state["issued"] += 1
                            while (state["dqi"] < len(dq)
                                   and state["dqi"] < state["issued"] - LAG
                                   and drain_ready()):
                                do_drain()
                while state["dqi"] < len(dq):
                    do_drain()

            # ---------------- Phase C: output projection ----------------
            def outproj(qb):
                ot_sb = ots[qb % 2]
                for qc in range(4):
                    for nb in range(8):
                        o_ps = ps.tile([128, 512], F32, tag="b", name="ops")
                        for h in range(QH):
                            nc.tensor.matmul(
                                o_ps,
                                ot_sb[h][:, qc * 128:(qc + 1) * 128],
                                wo_sb[:, h, nb * 512:(nb + 1) * 512],
                                start=(h == 0), stop=(h == QH - 1),
                            )
                        ob = obp.tile([128, 512], BF16, tag="ob", name="ob")
                        # halve eviction latency: both engines evict in
                        # parallel so the 2-slot o_ps rotation never stalls
                        nc.vector.tensor_copy(ob[:, 0:256], o_ps[:, 0:256])
                        nc.scalar.copy(ob[:, 256:512], o_ps[:, 256:512])
                        nc.sync.dma_start(
                            out=out[qb * 512 + qc * 128:
                                    qb * 512 + (qc + 1) * 128,
                                    nb * 512:(nb + 1) * 512],
                            in_=ob,
                        )

            attention(0)
            attention(1)
            outproj(0)
            attention(2)
            outproj(1)
            attention(3)
            outproj(2)
            outproj(3)
    nc.finalize()
    return nc


_NC_CACHE = {}


def _get_nc():
    if "nc" not in _NC_CACHE:
        _NC_CACHE["nc"] = build_nc()
    return _NC_CACHE["nc"]


def _host_prep(x, cos, sin, mask, wq, wk, wv, wo):
    xT = np.ascontiguousarray(x[0].T.astype(np.float32)).astype(NPBF)
    cosT = np.ascontiguousarray(cos[:, 0, :].T.astype(np.float32))
    sinT = sin[:, 0, :].T.astype(np.float32)
    sinTs = np.ascontiguousarray(
        np.concatenate([-sinT[:64], sinT[64:]], axis=0)
    )
    rr = np.arange(128, dtype=np.int64)[:, None]
    cc = np.arange(128, dtype=np.int64)[None, :]
    stair = np.where(rr <= cc, 0.0, NEG).astype(np.float32)
    cc9 = np.arange(896, dtype=np.int64)[None, :]
    stair9 = np.where(rr <= cc9 - 384, 0.0, NEG).astype(np.float32)
    ident = np.eye(128, dtype=np.float32)
    ones128 = np.ones((128, 128), dtype=np.float32).astype(NPBF)

    in_maps = []
    for i in range(N_CORES):
        in_maps.append({
            "xT": xT,
            "wq": np.ascontiguousarray(wq[:, i * QS:(i + 1) * QS]).astype(NPBF),
            "wk": np.ascontiguousarray(wk[:, i * 128:(i + 1) * 128]).astype(NPBF),
            "wv": np.ascontiguousarray(wv[:, i * 128:(i + 1) * 128]).astype(NPBF),
            "wo": np.ascontiguousarray(wo[i * QS:(i + 1) * QS, :]).astype(NPBF),
            "cosT": cosT,
            "sinTs": sinTs,
            "stair": stair,
            "stair9": stair9,
            "ident": ident,
            "ones128": ones128,
        })
    return in_maps


def kernel(x, cos, sin, mask, wq, wk, wv, wo, _trace=False, _trace_kwargs=None):
    nc = _get_nc()
    in_maps = _host_prep(x, cos, sin, mask, wq, wk, wv, wo)
    res = run_bass_kernel_spmd(
        nc, in_maps, list(range(N_CORES)), trace=_trace,
        **(_trace_kwargs or {}),
    )
    partials = [res.results[i]["out"].astype(np.float64)
                for i in range(N_CORES)]
    full = np.sum(np.stack(partials, axis=0), axis=0)
    out = full.astype(np.float32)[None, :, :]
    if _trace:
        return out, res
    return out
